# revision 32
# baseline (speedup 1.0000x reference)
"""MRI data-consistency pipelined-CG solver on 8 Trainium2 NeuronCores.

Sharding: pure data-parallel, 1 batch sample per core; the CG scalars are
global batch sums -> one tiny AllReduce per CG round, overlapped with the
next operator application via the Ghysels-Vanroose pipelined-CG recurrence
(mathematically identical iterates to plain CG, same matvec count).

Per coil, the centered 2D FFT / IFFT are chained PE matmuls against the
centered DFT matrix Fc (symmetric), with the complex maths fused into the
streamed operand: rhs = [Fr | Fi] / [-Fi | Fr] so one PSUM tile accumulates
[Re | Im] at once.  All matmul operands are fp16 (fp32 streams the PE at
2 cycles/col, bf16 at 1; PSUM accumulation stays fp32).

All 16 coils' csm stay resident in SBUF as fp16 -- loaded once, not per
iteration.  Field layout: each HxW field lives in one SBUF tile folded as
[128, NB*W] (block b holds rows [b*128, ...)); junk regions are kept at 0.
"""

import numpy as np

CG_ITER = 10

_nc_cache = {}
LAST_RESULT = None
H16 = np.float16
_LDW_PATCHED = False


def _enable_ldw_opt():
    """concourse pins walrus --enable-ldw-opt=false; without the pass every
    LDWEIGHTS serializes with the preceding matmul (~+110ns per MM, ~30% of
    this kernel's PE time).  Rewrite the flag for our compiles; birsim and
    the accuracy gate still validate the result."""
    global _LDW_PATCHED
    if _LDW_PATCHED:
        return
    import concourse.bass_utils as _bu

    _orig = _bu.run_command

    def _patched(cmd, *a, **kw):
        if isinstance(cmd, list):
            cmd = [
                "--enable-ldw-opt=true" if c == "--enable-ldw-opt=false" else c
                for c in cmd
            ]
        return _orig(cmd, *a, **kw)

    _bu.run_command = _patched
    _LDW_PATCHED = True


def _blocks(n):
    out = []
    r0 = 0
    while r0 < n:
        sz = min(128, n - r0)
        out.append((r0, sz))
        r0 += sz
    return out


def _centered_dft(n):
    # Columns of Fc = centered orthonormal DFT applied to unit vectors:
    # y = fftshift(fft(ifftshift(x))) = Fc @ x. Fc is symmetric for even n.
    eye = np.eye(n)
    Fc = np.fft.fftshift(
        np.fft.fft(np.fft.ifftshift(eye, axes=0), axis=0, norm="ortho"), axes=0
    )
    return Fc


def _segs(Hc):
    """Complex-stacked layout: the 2*Hc (Re then Im) rows of a field packed
    into ceil(2*Hc/128) SBUF partition-blocks, segment-wise so that every
    segment keeps partition == source-partition (h mod 128 for engine writes,
    PSUM m-block partition for evacuations).  Entries:
    (comp, mb, p0, p1, blk, h0): rows h0..h0+(p1-p0) of component comp
    (from PSUM m-block mb, partitions p0:p1) live at partitions p0:p1 of
    stacked block blk.  For Hc=320 one Im segment is partition-shifted; its
    producing matmul writes PSUM partitions 64:128 via tile_position."""
    if Hc % 128 == 0:
        nb = Hc // 128
        return (
            [("R", m, 0, 128, m, 128 * m) for m in range(nb)]
            + [("I", m, 0, 128, nb + m, 128 * m) for m in range(nb)]
        ), False
    assert Hc == 320, f"unsupported H={Hc}"
    return [
        ("R", 0, 0, 128, 0, 0),
        ("R", 1, 0, 128, 1, 128),
        ("R", 2, 0, 64, 2, 256),
        ("I", 0, 0, 64, 3, 0),
        ("I", 0, 64, 128, 2, 64),
        ("I", 1, 0, 64, 4, 128),
        ("I", 1, 64, 128, 3, 192),
        ("I", 2, 64, 128, 4, 256),
    ], True


def _stacked_G(A, B, s, Hc):
    # folded [128, NBS*W] matrix whose stacked-row (comp,h) is A[h] for Re
    # rows and s*B[h] for Im rows, in the _segs row order
    segs, _ = _segs(Hc)
    W = A.shape[1]
    NBS = 2 * Hc // 128
    G = np.zeros((128, NBS * W), np.float32)
    for comp, mb, p0, p1, blk, h0 in segs:
        n = p1 - p0
        rows = A[h0 : h0 + n] if comp == "R" else s * B[h0 : h0 + n]
        G[p0:p1, blk * W : (blk + 1) * W] = rows
    return G.astype(H16)


def _build(Hc, Wc, Cc, iters, n_cores):
    import concourse.bacc as bacc
    import concourse.mybir as mybir
    import concourse.tile as tile

    f32 = mybir.dt.float32
    f16 = mybir.dt.float16
    OP = mybir.AluOpType

    nc = bacc.Bacc(trn_type="TRN2", num_devices=n_cores)

    W2 = 2 * Wc
    NBS = 2 * Hc // 128
    assert 2 * Hc % 128 == 0
    SEGS, SHIFT_TAIL = _segs(Hc)
    FWS = NBS * Wc
    us = nc.dram_tensor("us_image", [2, Hc, Wc], f32, kind="ExternalInput")
    rec = nc.dram_tensor("reconstruction", [2, Hc, Wc], f32, kind="ExternalInput")
    maskd = nc.dram_tensor("maskn", [Hc, Wc], f16, kind="ExternalInput")
    maskh_d = nc.dram_tensor("maskh", [128, Wc], f16, kind="ExternalInput")
    csm_r_d = nc.dram_tensor("csm_r", [Cc, Hc, Wc], f16, kind="ExternalInput")
    csm_i_d = nc.dram_tensor("csm_i", [Cc, Hc, Wc], f16, kind="ExternalInput")
    mu_d = nc.dram_tensor("mu", [1], f32, kind="ExternalInput")
    r1_d = nc.dram_tensor("r1", [Hc, W2], f16, kind="ExternalInput")
    r2_d = nc.dram_tensor("r2", [Hc, W2], f16, kind="ExternalInput")
    gfr_d = nc.dram_tensor("gfr", [128, FWS], f16, kind="ExternalInput")
    gfi_d = nc.dram_tensor("gfi", [128, FWS], f16, kind="ExternalInput")
    gir_d = nc.dram_tensor("gir", [128, FWS], f16, kind="ExternalInput")
    gii_d = nc.dram_tensor("gii", [128, FWS], f16, kind="ExternalInput")
    out_d = nc.dram_tensor("out", [2, Hc, Wc], f32, kind="ExternalOutput")

    BL = _blocks(Hc)
    NB = len(BL)
    FW = NB * Wc
    FW2 = NB * W2

    with tile.TileContext(nc) as tc:
        with (
            tc.tile_pool(name="consts", bufs=1) as consts,
            tc.tile_pool(name="state", bufs=1) as state,
            tc.tile_pool(name="work", bufs=1) as work,
            tc.tile_pool(name="small", bufs=1) as small,
            tc.tile_pool(name="psA", bufs=3, space="PSUM") as psA,
            tc.tile_pool(name="psS", bufs=1, space="PSUM") as psS,
            tc.tile_pool(name="dram", bufs=4, space="DRAM") as dram,
        ):
            allt = []

            def T(pool, name, shape, dtype=f32):
                tl = pool.tile(shape, dtype, tag=name)
                if shape[0] == 128 and shape[1] >= 8:
                    allt.append(tl)
                return tl

            R1 = T(consts, "R1", [128, FW2], f16)
            R2 = T(consts, "R2", [128, FW2], f16)
            GFr = T(consts, "GFr", [128, FWS], f16)
            GFi = T(consts, "GFi", [128, FWS], f16)
            GIr = T(consts, "GIr", [128, FWS], f16)
            GIi = T(consts, "GIi", [128, FWS], f16)
            maskf = T(consts, "maskf", [128, FW], f16)
            maskh = T(consts, "maskh", [128, Wc], f16)
            cs_r = [T(consts, f"csr{c}", [128, FW], f16) for c in range(Cc)]
            cs_i = [T(consts, f"csi{c}", [128, FW], f16) for c in range(Cc)]
            ones_col = T(consts, "ones_col", [128, 1])
            ones_row = T(consts, "ones_row", [1, 128])
            mu_b = T(consts, "mu_b", [128, 1])
            mu_sb = T(consts, "mu_sb", [1, 1])

            x_r = T(state, "x_r", [128, FW])
            x_i = T(state, "x_i", [128, FW])
            r_r = T(state, "r_r", [128, FW])
            r_i = T(state, "r_i", [128, FW])
            w_r = T(state, "w_r", [128, FW])
            w_i = T(state, "w_i", [128, FW])
            p_r = T(state, "p_r", [128, FW])
            p_i = T(state, "p_i", [128, FW])
            s_r = T(state, "s_r", [128, FW])
            s_i = T(state, "s_i", [128, FW])
            z_r = T(state, "z_r", [128, FW])
            z_i = T(state, "z_i", [128, FW])
            n_r = T(state, "n_r", [128, FW])
            n_i = T(state, "n_i", [128, FW])

            wb_r = T(work, "wb_r", [128, FW], f16)
            wb_i = T(work, "wb_i", [128, FW], f16)
            pj1b = T(work, "pj1b", [128, FW], f16)
            pj2b = T(work, "pj2b", [128, FW], f16)
            u1t = T(work, "u1t", [128, FW], f16)
            u2t = T(work, "u2t", [128, FW], f16)
            tRa = T(work, "tRa", [128, FW], f16)
            tIa = T(work, "tIa", [128, FW], f16)
            cp = [[T(work, f"cp{s}{x}", [128, FW], f16) for x in "ri"] for s in (0, 1)]
            s1t = [T(work, f"s1t{s}", [128, FWS], f16) for s in (0, 1)]
            kmt = [T(work, f"kmt{s}", [128, FWS], f16) for s in (0, 1)]
            s3t = [T(work, f"s3t{s}", [128, FWS], f16) for s in (0, 1)]
            zz = [[T(work, f"zz{s}{x}", [128, FW], f16) for x in "ri"] for s in (0, 1)]
            A32 = T(work, "A32", [128, FW])
            B32 = T(work, "B32", [128, FW])

            partials = T(small, "partials", [128, 8])
            redsums = T(small, "redsums", [1, 8])
            asum_t = T(small, "asum", [1, 8])
            scl = T(small, "scl", [1, 16])
            alphas = T(small, "alphas", [1, 4])
            bc = T(small, "bc", [128, 4])
            gold = T(small, "gold", [1, 1])
            ainv = T(small, "ainv", [1, 1])

            v = nc.vector
            g = nc.gpsimd
            sc = nc.scalar
            STT = v.scalar_tensor_tensor
            TT = v.tensor_tensor

            # ---- init: zero everything (keeps folded junk regions at 0 so
            # full-tile elementwise ops and reductions stay correct and no
            # uninitialized-SBUF NaN can propagate through 0*x paths)
            for tl in allt:
                v.memset(tl, 0.0)
            v.memset(ones_col, 1.0)
            v.memset(ones_row, 1.0)

            def load_folded(dst, src2d):
                nbf = Hc // 128
                full = nbf * 128
                wsrc = src2d.shape[-1]
                if nbf:
                    nc.gpsimd.dma_start(
                        out=dst[:, 0 : nbf * wsrc].rearrange("p (b w) -> p b w", b=nbf),
                        in_=src2d[0:full, :].rearrange("(b p) w -> p b w", p=128),
                    )
                if full < Hc:
                    rem = Hc - full
                    nc.gpsimd.dma_start(
                        out=dst[:rem, nbf * wsrc : (nbf + 1) * wsrc],
                        in_=src2d[full:Hc, :],
                    )

            def store_folded(src, dst2d):
                nbf = Hc // 128
                full = nbf * 128
                if nbf:
                    nc.gpsimd.dma_start(
                        out=dst2d[0:full, :].rearrange("(b p) w -> p b w", p=128),
                        in_=src[:, 0 : nbf * Wc].rearrange("p (b w) -> p b w", b=nbf),
                    )
                if full < Hc:
                    rem = Hc - full
                    nc.gpsimd.dma_start(
                        out=dst2d[full:Hc, :],
                        in_=src[:rem, nbf * Wc : (nbf + 1) * Wc],
                    )

            nc.gpsimd.dma_start(out=mu_sb[:1, :1], in_=mu_d[None, :])
            for c in range(Cc):
                load_folded(cs_r[c], csm_r_d[c])
                load_folded(cs_i[c], csm_i_d[c])
            load_folded(R1, r1_d[:])
            load_folded(R2, r2_d[:])
            nc.gpsimd.dma_start(out=GFr[:, :], in_=gfr_d[:, :])
            nc.gpsimd.dma_start(out=GFi[:, :], in_=gfi_d[:, :])
            nc.gpsimd.dma_start(out=GIr[:, :], in_=gir_d[:, :])
            nc.gpsimd.dma_start(out=GIi[:, :], in_=gii_d[:, :])
            load_folded(maskf, maskd[:])
            nc.gpsimd.dma_start(out=maskh[:, :], in_=maskh_d[:, :])

            # mu broadcast to [128,1]
            psb = psS.tile([128, 8], f32, tag="sc")
            nc.tensor.matmul(
                psb[:, :1], lhsT=ones_row[:1, :128], rhs=mu_sb[:1, :1],
                start=True, stop=True,
            )
            sc.copy(out=mu_b[:, :1], in_=psb[:, :1])

            # r0 = us + mu*rec (staged through A32/B32 and n tiles)
            load_folded(A32, us[0])
            load_folded(B32, us[1])
            load_folded(n_r, rec[0])
            load_folded(n_i, rec[1])
            STT(out=r_r, in0=n_r, scalar=mu_b[:, :1], in1=A32,
                op0=OP.mult, op1=OP.add)
            STT(out=r_i, in0=n_i, scalar=mu_b[:, :1], in1=B32,
                op0=OP.mult, op1=OP.add)
            v.memset(n_r, 0.0)
            v.memset(n_i, 0.0)

            def lhs_pair(xr, xi):
                def f(k, ksz, m0, msz):
                    return (
                        xr[:ksz, k * Wc + m0 : k * Wc + m0 + msz],
                        xi[:ksz, k * Wc + m0 : k * Wc + m0 + msz],
                    )
                return f



            def pi_view(pi, msz, shift):
                # Im accumulator for the ragged last m-block goes to PSUM
                # partitions 64:128 (tile_position col-group 64) so its
                # evacuation into the stacked layout stays partition-aligned
                if shift and msz < 128:
                    return pi[64 : 64 + msz, :]
                return pi[:msz, :]

            def mm_stage_nat(get_lhs, Ra, Rb, consume, shift):
                # stage-1: natural-fold complex input (separate Re/Im tiles),
                # 4 real matmuls per (m,k); PSUM out capped at one bank.
                for m, (m0, msz) in enumerate(BL):
                    pr = psA.tile([128, Wc], f32, tag="mmr")
                    pi = psA.tile([128, Wc], f32, tag="mmi")
                    piv = pi_view(pi, msz, shift)
                    for k, (k0, ksz) in enumerate(BL):
                        lr, li = get_lhs(k, ksz, m0, msz)
                        nc.tensor.matmul(pr[:msz, :], lhsT=lr,
                                         rhs=Ra[:ksz, k * W2 : k * W2 + Wc],
                                         start=(k == 0), stop=False)
                        nc.tensor.matmul(piv, lhsT=lr,
                                         rhs=Ra[:ksz, k * W2 + Wc : (k + 1) * W2],
                                         start=(k == 0), stop=False)
                        nc.tensor.matmul(pr[:msz, :], lhsT=li,
                                         rhs=Rb[:ksz, k * W2 : k * W2 + Wc],
                                         start=False, stop=(k == NB - 1))
                        nc.tensor.matmul(piv, lhsT=li,
                                         rhs=Rb[:ksz, k * W2 + Wc : (k + 1) * W2],
                                         start=False, stop=(k == NB - 1))
                    consume(m, m0, msz, pr, pi)

            def mm_stage_stk(xf, Gr, Gi, consume, shift):
                # stages 2-4: complex-stacked input tile [128, NBS*Wc]; all
                # contractions are full K=128, 2 matmuls per (m, kblock).
                for m, (m0, msz) in enumerate(BL):
                    pr = psA.tile([128, Wc], f32, tag="mmr")
                    pi = psA.tile([128, Wc], f32, tag="mmi")
                    piv = pi_view(pi, msz, shift)
                    for kb in range(NBS):
                        lh = xf[:, kb * Wc + m0 : kb * Wc + m0 + msz]
                        nc.tensor.matmul(pr[:msz, :], lhsT=lh,
                                         rhs=Gr[:, kb * Wc : (kb + 1) * Wc],
                                         start=(kb == 0), stop=(kb == NBS - 1))
                        nc.tensor.matmul(piv, lhsT=lh,
                                         rhs=Gi[:, kb * Wc : (kb + 1) * Wc],
                                         start=(kb == 0), stop=(kb == NBS - 1))
                    consume(m, m0, msz, pr, pi)

            def evac_stk(dst, masked):
                # PSUM pair -> complex-stacked tile per the SEGS table; all
                # pieces partition-aligned. ACT for plain copies, DVE for the
                # mask-fused stage-2 variant.
                def f(m, m0, msz, pr, pi):
                    for comp, mb, p0, p1, blk, h0 in SEGS:
                        if mb != m:
                            continue
                        src = pr if comp == "R" else pi
                        o = dst[p0:p1, blk * Wc : (blk + 1) * Wc]
                        if not masked:
                            sc.copy(out=o, in_=src[p0:p1, :])
                        elif (h0 % 128) != p0:
                            v.tensor_tensor(out=o, in0=src[p0:p1, :],
                                            in1=maskh[p0:p1, :], op=OP.mult)
                        else:
                            v.tensor_tensor(
                                out=o, in0=src[p0:p1, :],
                                in1=maskf[p0:p1, m * Wc : (m + 1) * Wc],
                                op=OP.mult)
                return f

            def evac_zz(slot):
                def f(m, m0, msz, pr, pi):
                    sc.copy(out=zz[slot][0][:msz, m * Wc : (m + 1) * Wc],
                            in_=pr[:msz, :])
                    sc.copy(out=zz[slot][1][:msz, m * Wc : (m + 1) * Wc],
                            in_=pi[:msz, :])
                return f

            def proj(c, dve_only=False):
                # cp = (wb) * csm_c; steady-state coils split DVE/GPS, the
                # iteration-tail prefix runs all-DVE (GPS fp16 mult is ~3us
                # and would sit on the critical path to the next S1)
                s = c % 2
                e2 = v if dve_only else g
                TT(out=pj1b, in0=wb_r, in1=cs_r[c], op=OP.mult)
                e2.tensor_tensor(out=pj2b, in0=wb_i, in1=cs_i[c], op=OP.mult)
                TT(out=cp[s][0], in0=pj1b, in1=pj2b, op=OP.subtract)
                e2.tensor_tensor(out=pj1b, in0=wb_r, in1=cs_i[c], op=OP.mult)
                TT(out=pj2b, in0=wb_i, in1=cs_r[c], op=OP.mult)
                e2.tensor_tensor(out=cp[s][1], in0=pj1b, in1=pj2b, op=OP.add)

            def mv_prefix(src_r, src_i):
                sc.copy(out=wb_r, in_=src_r)
                sc.copy(out=wb_i, in_=src_i)
                proj(0, dve_only=True)
                if Cc > 1:
                    proj(1, dve_only=True)

            def matvec(dst_r, dst_i, src_r, src_i, skip_prefix=False):
                # dst = A(src): per coil S*F^H M F S + mu*I, coils pipelined
                # two-deep so the PE never waits on a stage-boundary evac.
                if not skip_prefix:
                    mv_prefix(src_r, src_i)
                v.tensor_scalar_mul(out=dst_r, in0=src_r, scalar1=mu_b[:, :1])
                v.tensor_scalar_mul(out=dst_i, in0=src_i, scalar1=mu_b[:, :1])

                def combine(c, paired):
                    # coil contribution zz*conj(csm) folded in fp16, pairs of
                    # coils merged before the expensive fp32 accumulate
                    s = c % 2
                    if paired and s == 0:
                        TT(out=u1t, in0=zz[0][0], in1=cs_r[c], op=OP.mult)
                        TT(out=u2t, in0=zz[0][1], in1=cs_i[c], op=OP.mult)
                        TT(out=tRa, in0=u1t, in1=u2t, op=OP.add)
                        TT(out=u1t, in0=zz[0][1], in1=cs_r[c], op=OP.mult)
                        TT(out=u2t, in0=zz[0][0], in1=cs_i[c], op=OP.mult)
                        TT(out=tIa, in0=u1t, in1=u2t, op=OP.subtract)
                    else:
                        # disjoint scratch per half: the fp32 accumulates run
                        # on GPSIMD (off DVE, so they can't delay the next
                        # stage's mask evacuations) and nothing DVE writes
                        # next overlaps what GPSIMD still reads
                        TT(out=u1t, in0=zz[s][0], in1=cs_r[c], op=OP.mult)
                        TT(out=u2t, in0=zz[s][1], in1=cs_i[c], op=OP.mult)
                        TT(out=u1t, in0=u1t, in1=u2t, op=OP.add)
                        if paired:
                            TT(out=u1t, in0=u1t, in1=tRa, op=OP.add)
                        g.tensor_tensor(out=dst_r, in0=dst_r, in1=u1t,
                                        op=OP.add)
                        TT(out=u2t, in0=zz[s][1], in1=cs_r[c], op=OP.mult)
                        TT(out=tRa, in0=zz[s][0], in1=cs_i[c], op=OP.mult)
                        TT(out=u2t, in0=u2t, in1=tRa, op=OP.subtract)
                        if paired:
                            TT(out=u2t, in0=u2t, in1=tIa, op=OP.add)
                        g.tensor_tensor(out=dst_i, in0=dst_i, in1=u2t,
                                        op=OP.add)

                def S1(c):
                    s = c % 2
                    mm_stage_nat(lhs_pair(cp[s][0], cp[s][1]), R1, R2,
                                 evac_stk(s1t[s], masked=False), SHIFT_TAIL)

                def S2(c):
                    mm_stage_stk(s1t[c % 2], GFr, GFi,
                                 evac_stk(kmt[c % 2], masked=True), SHIFT_TAIL)

                def S3(c):
                    mm_stage_stk(kmt[c % 2], GIr, GIi,
                                 evac_stk(s3t[c % 2], masked=False), SHIFT_TAIL)

                def S4(c):
                    mm_stage_stk(s3t[c % 2], GIr, GIi, evac_zz(c % 2), False)

                for base in range(0, Cc, 2):
                    a = base
                    b = base + 1 if base + 1 < Cc else None
                    S1(a)
                    if b is not None:
                        S1(b)
                    if base + 2 < Cc:
                        proj(base + 2)
                        if base + 3 < Cc:
                            proj(base + 3)
                    S2(a)
                    if b is not None:
                        S2(b)
                    S3(a)
                    if b is not None:
                        S3(b)
                    S4(a)
                    if b is not None:
                        S4(b)
                    combine(a, paired=(b is not None))
                    if b is not None:
                        combine(b, paired=True)

            def reduction_round(k):
                # redsums[0, :k] already holds the local sums
                din = dram.tile([1, 8], f32, tag="cin")
                dout = dram.tile([1, 8], f32, tag="cout")
                nc.gpsimd.dma_start(out=din[:1, :k], in_=redsums[:1, :k])
                if n_cores > 1:
                    nc.gpsimd.collective_compute(
                        "AllReduce", OP.add,
                        replica_groups=[list(range(n_cores))],
                        ins=[din[:1, :k].opt()],
                        outs=[dout[:1, :k].opt()],
                    )
                else:
                    nc.gpsimd.dma_start(out=dout[:1, :k], in_=din[:1, :k])
                nc.gpsimd.dma_start(out=asum_t[:1, :k], in_=dout[:1, :k])

            def axpy(out, in0, scalar, in1):
                # out = in0*scalar + in1 (TensorScalarPtr is DVE-only ISA)
                STT(out=out, in0=in0, scalar=scalar, in1=in1,
                    op0=OP.mult, op1=OP.add)

            # ---- w0 = A r0
            matvec(w_r, w_i, r_r, r_i)

            for it in range(iters):
                last = it == iters - 1
                # gamma = sum |r|^2 ; delta = sum Re(w conj(r)) -- entirely
                # on GPSIMD (full XYZWC reduce straight to [1,1]): latency
                # hides behind the matvec and DVE stays free for the
                # iteration-tail critical path
                g.tensor_tensor(out=A32, in0=r_r, in1=r_r, op=OP.mult)
                g.reduce_sum(out=redsums[:1, 0:1], in_=A32,
                             axis=mybir.AxisListType.XYZWC)
                g.tensor_tensor(out=A32, in0=r_i, in1=r_i, op=OP.mult)
                g.reduce_sum(out=redsums[:1, 1:2], in_=A32,
                             axis=mybir.AxisListType.XYZWC)
                g.tensor_tensor(out=B32, in0=w_r, in1=r_r, op=OP.mult)
                g.reduce_sum(out=redsums[:1, 2:3], in_=B32,
                             axis=mybir.AxisListType.XYZWC)
                g.tensor_tensor(out=B32, in0=w_i, in1=r_i, op=OP.mult)
                g.reduce_sum(out=redsums[:1, 3:4], in_=B32,
                             axis=mybir.AxisListType.XYZWC)
                reduction_round(4)

                # n = A w overlaps the AllReduce latency (skipped last iter:
                # x only needs alpha/beta there). For it>0 the cast+proj
                # prefix was already emitted right after the last w update,
                # so the PE restarts without waiting on the full update tail.
                if not last:
                    matvec(n_r, n_i, w_r, w_i, skip_prefix=(it > 0))

                # scalars: gamma=c0+c1, delta=c2+c3
                TT(out=scl[:1, 0:1], in0=asum_t[:1, 0:1], in1=asum_t[:1, 1:2],
                   op=OP.add)
                TT(out=scl[:1, 1:2], in0=asum_t[:1, 2:3], in1=asum_t[:1, 3:4],
                   op=OP.add)
                if it == 0:
                    v.memset(alphas[:1, 0:1], 0.0)  # beta = 0
                    v.reciprocal(out=scl[:1, 6:7], in_=scl[:1, 1:2])
                    TT(out=alphas[:1, 1:2], in0=scl[:1, 0:1], in1=scl[:1, 6:7],
                       op=OP.mult)  # alpha = gamma/delta
                else:
                    v.reciprocal(out=scl[:1, 2:3], in_=gold[:1, :1])
                    TT(out=alphas[:1, 0:1], in0=scl[:1, 0:1], in1=scl[:1, 2:3],
                       op=OP.mult)  # beta = gamma/gamma_old
                    TT(out=scl[:1, 3:4], in0=scl[:1, 0:1], in1=ainv[:1, :1],
                       op=OP.mult)  # gamma/alpha_old
                    TT(out=scl[:1, 4:5], in0=alphas[:1, 0:1], in1=scl[:1, 3:4],
                       op=OP.mult)  # beta*gamma/alpha_old
                    TT(out=scl[:1, 5:6], in0=scl[:1, 1:2], in1=scl[:1, 4:5],
                       op=OP.subtract)  # delta - beta*gamma/alpha_old
                    v.reciprocal(out=scl[:1, 6:7], in_=scl[:1, 5:6])
                    TT(out=alphas[:1, 1:2], in0=scl[:1, 0:1], in1=scl[:1, 6:7],
                       op=OP.mult)  # alpha
                v.tensor_scalar_mul(out=alphas[:1, 2:3], in0=alphas[:1, 1:2],
                                    scalar1=-1.0)
                sc.copy(out=gold[:1, :1], in_=scl[:1, 0:1])
                v.reciprocal(out=ainv[:1, :1], in_=alphas[:1, 1:2])

                psb2 = psS.tile([128, 8], f32, tag="sc")
                nc.tensor.matmul(psb2[:, :3], lhsT=ones_row[:1, :128],
                                 rhs=alphas[:1, :3], start=True, stop=True)
                sc.copy(out=bc[:, :3], in_=psb2[:, :3])
                bet = bc[:, 0:1]
                alp = bc[:, 1:2]
                nal = bc[:, 2:3]

                # updates: z,s then w first so the next matvec's cast+proj
                # prefix launches while p,x,r still run; s uses pre-update w
                if last:
                    axpy(p_r, p_r, bet, r_r)
                    axpy(p_i, p_i, bet, r_i)
                    axpy(x_r, p_r, alp, x_r)
                    axpy(x_i, p_i, alp, x_i)
                else:
                    axpy(z_r, z_r, bet, n_r)
                    axpy(z_i, z_i, bet, n_i)
                    axpy(s_r, s_r, bet, w_r)
                    axpy(s_i, s_i, bet, w_i)
                    axpy(w_r, z_r, nal, w_r)
                    axpy(w_i, z_i, nal, w_i)
                    if it + 1 < iters - 1:
                        # prefix of the NEXT iteration's matvec
                        mv_prefix(w_r, w_i)
                    axpy(p_r, p_r, bet, r_r)
                    axpy(p_i, p_i, bet, r_i)
                    axpy(x_r, p_r, alp, x_r)
                    axpy(x_i, p_i, alp, x_i)
                    axpy(r_r, s_r, nal, r_r)
                    axpy(r_i, s_i, nal, r_i)

            store_folded(x_r, out_d[0])
            store_folded(x_i, out_d[1])

    nc.compile()
    return nc


def kernel(us_image, reconstruction, mask, csm_r, csm_i, mu):
    global LAST_RESULT
    from concourse.bass_utils import run_bass_kernel_spmd

    Bc, _, Hc, Wc = us_image.shape
    Cc = csm_r.shape[1]
    n_cores = Bc
    iters = CG_ITER

    key = (Hc, Wc, Cc, iters, n_cores)
    if key not in _nc_cache:
        _nc_cache[key] = _build(Hc, Wc, Cc, iters, n_cores)
    nc = _nc_cache[key]

    Fc = _centered_dft(Hc)
    Fr = np.ascontiguousarray(Fc.real).astype(np.float32)
    Fi = np.ascontiguousarray(Fc.imag).astype(np.float32)
    r1 = np.concatenate([Fr, Fi], axis=1).astype(H16)
    r2 = np.concatenate([-Fi, Fr], axis=1).astype(H16)
    gfr = _stacked_G(Fr, Fi, -1.0, Hc)   # FFT  Re-out: Xr*Fr - Xi*Fi
    gfi = _stacked_G(Fi, Fr, +1.0, Hc)   # FFT  Im-out: Xr*Fi + Xi*Fr
    gir = _stacked_G(Fr, Fi, +1.0, Hc)   # IFFT Re-out: Xr*Fr + Xi*Fi
    gii = _stacked_G(-Fi, Fr, +1.0, Hc)  # IFFT Im-out: -Xr*Fi + Xi*Fr

    in_maps = []
    for b in range(n_cores):
        mb16 = mask[b, 0].astype(H16)
        mh = np.zeros((128, Wc), H16)
        if Hc % 128:
            tail = Hc - (Hc // 128) * 128
            mh[128 - tail :, :] = mb16[Hc - tail :, :]
        in_maps.append(
            {
                "us_image": np.ascontiguousarray(us_image[b], dtype=np.float32),
                "reconstruction": np.ascontiguousarray(
                    reconstruction[b], dtype=np.float32
                ),
                "maskn": np.ascontiguousarray(mb16),
                "maskh": mh,
                "csm_r": np.ascontiguousarray(csm_r[b].astype(H16)),
                "csm_i": np.ascontiguousarray(csm_i[b].astype(H16)),
                "mu": np.ascontiguousarray(mu, dtype=np.float32),
                "r1": r1,
                "r2": r2,
                "gfr": gfr,
                "gfi": gfi,
                "gir": gir,
                "gii": gii,
            }
        )

    res = run_bass_kernel_spmd(nc, in_maps, core_ids=list(range(n_cores)))
    LAST_RESULT = res
    out = np.stack([res.results[b]["out"] for b in range(n_cores)], axis=0)
    return out.astype(np.float32)
